# revision 11
# baseline (speedup 1.0000x reference)
"""LN + Linear (no bias) + Sigmoid, tensor-parallel over 8 TRN2 NeuronCores.

Math: y = sigmoid(LN(x) @ W.T), x [8192, 4096] f32, W [16384, 4096] f32.

Sharding: column-parallel — W sharded along d_out into 8 shards of 2048,
x replicated; no collectives. Per core the device kernel does, per token
tile of 128:

    1. DMA x tile in natural layout (bf16), LN stats via bn_stats/bn_aggr,
       r = rsqrt(var+eps), then one fused ScalarE op
       xh = Identity(x*r + (-mean*r)) -> normalized tile (bf16).
    2. One SBUF->SBUF DMA-transpose (X-bar) turns the [128t, 4096d] tile
       into the K-major stationary layout xb[p, k, t] = xh[t, d(p,k)].
       The d index ordering the xbar produces is absorbed into the host-side
       layout of W^T, so the contraction pairs up correctly whatever it is.
    3. GEMM: out[t,o] = sum_d xh[t,d] W[o,d]; per k-tile one stationary load
       serves four 512-col matmuls into a 4-bank [128, 2048] f32 PSUM
       accumulation group (double-buffered across token tiles).
    4. Sigmoid at PSUM eviction (ScalarE) -> bf16 -> DMA to DRAM.

Only one copy of x ships to each core and the output returns as bf16;
kernel() upcasts to f32 on host.

build(repeat=K) wraps the whole pipeline in a hardware For_i loop that
executes the identical program K times back-to-back on device — used by
test.py to measure the marginal per-iteration HW execution time with the
dispatch/transfer overhead of the remote tunnel cancelled out.
"""

import os

import numpy as np
import ml_dtypes

T = 8192        # tokens
D = 4096        # d_in (contraction)
O_FULL = 16384  # d_out
NCORES = 8
OSH = O_FULL // NCORES  # 2048 per-core output shard
P = 128
NK = D // P     # 32 k-tiles
NT = T // P     # 64 token tiles
EPS = 1e-5

# d-index ordering produced by the X-bar transpose into xb[p, k, t]:
#   "p" -> d = p * NK + k (p-major), "k" -> d = k * P + p (k-major).
# Verified on hardware; the W host layout below must match.
XBAR_D_ORDER = os.environ.get("XBAR_D_ORDER", "k")

_BUILT = None
LAST_RESULTS = None


def build(repeat=1):
    import concourse.bass as bass
    import concourse.mybir as mybir
    import concourse.tile as tile
    from concourse import bacc

    f32 = mybir.dt.float32
    bf16 = mybir.dt.bfloat16
    AF = mybir.ActivationFunctionType

    nc = bacc.Bacc("TRN2", target_bir_lowering=False, debug=False,
                   num_devices=NCORES)

    xn_d = nc.dram_tensor("xn", [T, D], bf16, kind="ExternalInput")
    # wt[k, p, o] = W_shard[o, d(p, k)] — see XBAR_D_ORDER
    wt_d = nc.dram_tensor("wt", [NK, P, OSH], bf16, kind="ExternalInput")
    out_d = nc.dram_tensor("out", [T, OSH], bf16, kind="ExternalOutput")

    nt_work = int(os.environ.get("NT_WORK", NT))  # debug knob

    with tile.TileContext(nc) as tc:
        with (
            tc.tile_pool(name="const", bufs=1) as const,
            tc.tile_pool(name="xn", bufs=2) as xnpool,
            tc.tile_pool(name="xh", bufs=2) as xhpool,
            tc.tile_pool(name="xb", bufs=2) as xbpool,
            tc.tile_pool(name="st", bufs=2) as stpool,
            tc.tile_pool(name="vec", bufs=2) as vecpool,
            tc.tile_pool(name="ot", bufs=2) as otpool,
            tc.tile_pool(name="ps", bufs=2, space="PSUM") as pspool,
        ):
            eps_sb = const.tile([P, 1], f32)
            nc.vector.memset(eps_sb[:, :], EPS)

            w_sb = const.tile([P, NK, OSH], bf16)
            for k in range(NK):
                nc.sync.dma_start(out=w_sb[:, k, :], in_=wt_d[k])

            state = {}

            def emit_dma(i):
                xnb = xnpool.tile([P, D], bf16)
                nc.sync.dma_start(out=xnb[:, :], in_=xn_d[i * P:(i + 1) * P, :])
                state[i] = {"xnb": xnb}

            def emit_stats_norm(i):
                s = state[i]
                xnb = s["xnb"]
                xs3 = xnb[:, :].rearrange("p (n f) -> p n f", f=512)
                stats = stpool.tile([P, D // 512, 6], f32)
                for j in range(D // 512):
                    nc.vector.bn_stats(out=stats[:, j, :], in_=xs3[:, j, :])
                mv = stpool.tile([P, 2], f32)
                nc.vector.bn_aggr(out=mv[:, :], in_=stats[:, :, :])
                std = stpool.tile([P, 1], f32)
                nc.scalar.activation(std[:, :], mv[:, 1:2], AF.Sqrt,
                                     bias=eps_sb[:, :])
                r = vecpool.tile([P, 1], f32)
                nc.vector.reciprocal(r[:, :], std[:, :])
                nm = vecpool.tile([P, 1], f32)
                nc.scalar.mul(nm[:, :], mv[:, 0:1], -1.0)
                nbias = vecpool.tile([P, 1], f32)
                nc.vector.tensor_mul(nbias[:, :], nm[:, :], r[:, :])
                xh = xhpool.tile([P, D], bf16)
                nc.scalar.activation(xh[:, :], xnb[:, :], AF.Identity,
                                     bias=nbias[:, :], scale=r[:, :])
                # X-bar transpose into the K-major GEMM stationary layout
                xb = xbpool.tile([P, NK, P], bf16, name="xb", tag="xb")
                nc.sync.dma_start(out=xb[:, :, :], in_=xh[:, :], transpose=True)
                s["xb"] = xb

            def emit_gemm(i):
                s = state[i]
                ps = pspool.tile([P, OSH], f32, tag="ps")
                xb = s["xb"]
                for k in range(NK):
                    st, sp = (k == 0), (k == NK - 1)
                    for q in range(4):
                        nc.tensor.matmul(ps[:, q * 512:(q + 1) * 512],
                                         xb[:, k, :],
                                         w_sb[:, k, q * 512:(q + 1) * 512],
                                         start=st, stop=sp)
                ot = otpool.tile([P, OSH], bf16)
                nc.scalar.activation(ot[:, 0:1024], ps[:, 0:1024], AF.Sigmoid)
                nc.scalar.activation(ot[:, 1024:2048], ps[:, 1024:2048],
                                     AF.Sigmoid)
                nc.sync.dma_start(out=out_d[i * P:(i + 1) * P, :],
                                  in_=ot[:, :])

            def emit_all():
                emit_dma(0)
                emit_stats_norm(0)
                for i in range(nt_work):
                    if i + 1 < nt_work:
                        emit_dma(i + 1)
                        emit_stats_norm(i + 1)
                    emit_gemm(i)
                    del state[i]

            if repeat == 1:
                emit_all()
            elif repeat % 2 == 0:
                # Two bodies per loop iteration: halves the per-iteration
                # all-engine barrier + pipeline-drain cost (~25 us).
                with tc.For_i(0, repeat // 2) as _:
                    emit_all()
                    emit_all()
            else:
                with tc.For_i(0, repeat) as _:
                    emit_all()

    nc.compile()
    return nc


def _get_nc():
    global _BUILT
    if _BUILT is None:
        _BUILT = build(repeat=1)
    return _BUILT


def prepare_in_maps(x, W):
    x = np.asarray(x, dtype=np.float32)
    W = np.asarray(W, dtype=np.float32)
    bf = ml_dtypes.bfloat16

    xn = x.astype(bf)
    in_maps = []
    for c in range(NCORES):
        Wsh = W[c * OSH:(c + 1) * OSH]                    # [2048, 4096]
        WshT = np.ascontiguousarray(Wsh.T)               # [4096, 2048] d-major
        if XBAR_D_ORDER == "k":
            wt = WshT.reshape(NK, P, OSH)                # d = k*128 + p
        else:
            wt = np.ascontiguousarray(
                WshT.reshape(P, NK, OSH).transpose(1, 0, 2))  # d = p*32 + k
        in_maps.append({"xn": xn, "wt": wt.astype(bf)})
    return in_maps


def kernel(x, W):
    global LAST_RESULTS
    from concourse.bass_utils import run_bass_kernel_spmd

    in_maps = prepare_in_maps(x, W)
    nc = _get_nc()
    res = run_bass_kernel_spmd(nc, in_maps, list(range(NCORES)))
    LAST_RESULTS = res
    out = np.concatenate(
        [np.asarray(res.results[c]["out"]) for c in range(NCORES)], axis=1)
    return np.ascontiguousarray(out).astype(np.float32)
